# revision 5
# baseline (speedup 1.0000x reference)
"""Bass/Trainium2 kernel for nn_EquivSetGNN3 (gnn_message_passing).

Math (reference): x = relu(x@W_in+b_in); x0 = x
  2 layers of: Xe = segsum_E((x@W1+b1)[V]); Xev = cat(x[V], Xe[E])@W2+b2
               Xv = segsum_V(Xev); x = relu((0.5*Xv + 0.5*x0)@W3 + b3)

Algebraic restructuring (all weight/bias work folded out of the nnz path):
  XeRaw = segsum_E x[V]                      (phase A: pure segment sum)
  SB    = segsum_V XeRaw[E]                  (phase B: pure segment sum)
  Xv    = degV (*) x @ W2a + SB @ (W1 W2b) + cvec
  cvec  = wdegV (x) (b1 W2b) + degV (x) b2   (host precomputed, [N, C])
  x'    = relu(0.5 (Xv + x0) @ W3 + b3)
where wdegV[v] = sum_{(v,e)} degE[e].

Segment sums: dma_gather of 256B f16 rows + one-hot matmuls (P in fp8) on
the TensorEngine, accumulated in PSUM per 128-segment superchunk.
Gathers are batched into few large calls (SWDGE desc-gen on the gpsimd Q7
was the baseline bottleneck at ~2.2us per call).

Sharding: nodes and edges split 8 ways (graph parallel); x and XeRaw are
AllGathered between phases; weights replicated. DRAM feature tensors use a
partition-major [128, n_sc*C] layout so all stores are single batched DMAs;
gather indices are host-remapped into that layout.
"""
import numpy as np
import ml_dtypes

import concourse.bacc as bacc
import concourse.mybir as mybir
import concourse.tile as tile
from concourse.bass_utils import run_bass_kernel_spmd

f32 = mybir.dt.float32
f16 = mybir.dt.float16
bf16 = mybir.dt.bfloat16
f8 = mybir.dt.float8e4
i16 = mybir.dt.int16

SEG_DT = f16                     # gathered-feature dtype
P_DT = f8                        # one-hot matrix dtype
P_NP = ml_dtypes.float8_e4m3
BF_NP = ml_dtypes.bfloat16

N = 50000
M = 25000
C = 128
R = 8
NO = N // R                      # 6250 nodes per core
EO = M // R                      # 3125 edges per core
SCB = (NO + 127) // 128          # 49 node superchunks per core
SCA = (EO + 127) // 128          # 25 edge superchunks per core
NPAD = R * 128 * SCB             # 50176 padded x_full rows
MPAD = R * 128 * SCA             # 25600 padded xe_full rows
SPLIT = 32768                    # int16 positive range limit for gather idxs
GA = 3                           # phase A superchunks per gather group
GB = 6                           # phase B superchunks per gather group
GP = 7                           # prologue superchunks per load group
N_LAYERS = 2

_cache = {}


def _node_gidx(v):
    """global node id -> row in partition-major padded x_full."""
    r, o = np.divmod(v, NO)
    s, p = np.divmod(o, 128)
    return r * (128 * SCB) + p * SCB + s


def _edge_gidx(e):
    """global edge id -> row in partition-major padded xe_full."""
    r, o = np.divmod(e, EO)
    s, p = np.divmod(o, 128)
    return r * (128 * SCA) + p * SCA + s


def _wrap_idx(flat):
    """[L] int -> [128, L//16] int16 (idx i at partition i%16, col i//16;
    replicated 8x across partition groups for the 8 gpsimd cores)."""
    w = flat.reshape(-1, 16).T.astype(np.int16)
    return np.ascontiguousarray(np.tile(w, (8, 1)))


def _make_P(flat):
    """[L] rel ids -> one-hot P [128, L] fp8: P[p, c*128+j] = (rel[c*128+p]==j).
    Padded entries (rel == -1) give all-zero rows."""
    rel = flat.reshape(-1, 128)          # [nch, 128] (chunk, partition)
    oh = rel[:, :, None] == np.arange(128)[None, None, :]  # [nch, p, j]
    return np.ascontiguousarray(
        oh.transpose(1, 0, 2).reshape(128, -1).astype(P_NP))


def _pad_to(arr, n, val):
    out = np.full(n, val, dtype=np.int64)
    out[: len(arr)] = arr
    return out


def _groups(n_sc, g):
    return [list(range(s, min(s + g, n_sc))) for s in range(0, n_sc, g)]


def _prepare(V, E):
    """Host-side preprocessing: sorted/sharded/padded gather index+rel arrays.

    Returns (meta, per_core). Chunk layout per gather group: all bucket-A
    chunks (superchunk-major), then all bucket-B chunks."""
    gV = _node_gidx(V)
    # ---- phase A: incidences sorted by E (edge-major) ----
    oA = np.argsort(E, kind="stable")
    Va, Ea = gV[oA], E[oA]
    e0 = np.arange(R)[:, None] * EO + np.arange(SCA)[None, :] * 128  # [R,SCA]
    e1 = np.minimum(e0 + 128, (np.arange(R)[:, None] + 1) * EO)
    lo = np.searchsorted(Ea, e0.ravel()).reshape(R, SCA)
    hi = np.searchsorted(Ea, e1.ravel()).reshape(R, SCA)

    cntA = np.zeros((R, SCA), np.int64)
    cntB = np.zeros((R, SCA), np.int64)
    for r in range(R):
        for s in range(SCA):
            seg = Va[lo[r, s]: hi[r, s]]
            nb = int((seg >= SPLIT).sum())
            cntB[r, s] = nb
            cntA[r, s] = len(seg) - nb
    nchA = (-(-cntA.max(0) // 128)).astype(np.int64)  # [SCA]
    nchB = (-(-cntB.max(0) // 128)).astype(np.int64)

    # ---- phase B: incidences sorted by V (node-major) ----
    gE = _edge_gidx(E)
    oB = np.argsort(V, kind="stable")
    Vb, Eb = V[oB], gE[oB]
    v0 = np.arange(R)[:, None] * NO + np.arange(SCB)[None, :] * 128
    v1 = np.minimum(v0 + 128, (np.arange(R)[:, None] + 1) * NO)
    lo2 = np.searchsorted(Vb, v0.ravel()).reshape(R, SCB)
    hi2 = np.searchsorted(Vb, v1.ravel()).reshape(R, SCB)
    cnt2 = hi2 - lo2
    nch2 = (-(-cnt2.max(0) // 128)).astype(np.int64)  # [SCB]

    meta = {
        "nchA": nchA.tolist(),
        "nchB": nchB.tolist(),
        "nch2": nch2.tolist(),
    }

    degE = np.bincount(E, minlength=M).astype(np.float64)
    degV = np.bincount(V, minlength=N).astype(np.float64)
    wdegV = np.zeros(N, np.float64)
    np.add.at(wdegV, V, degE[E])

    per_core = []
    for r in range(R):
        idxA_parts, relA_parts = [], []
        for scs in _groups(SCA, GA):
            aidx, arel, bidx, brel = [], [], [], []
            for s in scs:
                seg_v = Va[lo[r, s]: hi[r, s]]
                seg_e = Ea[lo[r, s]: hi[r, s]] - e0[r, s]
                mB = seg_v >= SPLIT
                la = int(nchA[s]) * 128
                aidx.append(_pad_to(seg_v[~mB], la, 0))
                arel.append(_pad_to(seg_e[~mB], la, -1))
                lb = int(nchB[s]) * 128
                bidx.append(_pad_to(seg_v[mB] - SPLIT, lb, 0))
                brel.append(_pad_to(seg_e[mB], lb, -1))
            idxA_parts += aidx + bidx
            relA_parts += arel + brel
        idxA = np.concatenate(idxA_parts)
        relA = np.concatenate(relA_parts)

        idxB_parts, relB_parts = [], []
        for scs in _groups(SCB, GB):
            for s in scs:
                seg_e = Eb[lo2[r, s]: hi2[r, s]]
                seg_v = Vb[lo2[r, s]: hi2[r, s]] - v0[r, s]
                lb = int(nch2[s]) * 128
                idxB_parts.append(_pad_to(seg_e, lb, 0))
                relB_parts.append(_pad_to(seg_v, lb, -1))
        idxB = np.concatenate(idxB_parts)
        relB = np.concatenate(relB_parts)

        per_core.append(
            {
                "idxA": _wrap_idx(idxA),
                "PA": _make_P(relA),
                "idxB": _wrap_idx(idxB),
                "PB": _make_P(relB),
                "degV": degV[r * NO: (r + 1) * NO],
                "wdegV": wdegV[r * NO: (r + 1) * NO],
            }
        )
    return meta, per_core


def _build(meta):
    nchA = meta["nchA"]
    nchB = meta["nchB"]
    nch2 = meta["nch2"]
    LA = sum(a + b for a, b in zip(nchA, nchB)) * 128
    LB = sum(nch2) * 128

    nc = bacc.Bacc("TRN2", target_bir_lowering=False, debug=False,
                   num_devices=R, num_swdge_queues=4)

    # ---- kernel I/O ----
    xsh = nc.declare_dram_parameter("xsh", [128, SCB * C], f32, isOutput=False)
    cv_d = nc.declare_dram_parameter("cv", [C, NO], f32, isOutput=False)
    dv_d = nc.declare_dram_parameter("dv", [C, NO], bf16, isOutput=False)
    w_in = nc.declare_dram_parameter("w_in", [C, C], bf16, isOutput=False)
    w2a = nc.declare_dram_parameter("w2a", [C, C], bf16, isOutput=False)
    w2bp = nc.declare_dram_parameter("w2bp", [C, C], bf16, isOutput=False)
    w3h = nc.declare_dram_parameter("w3h", [C, C], bf16, isOutput=False)
    b_in = nc.declare_dram_parameter("b_in", [C, 1], f32, isOutput=False)
    b3d = nc.declare_dram_parameter("b3", [C, 1], f32, isOutput=False)
    idxA_d = nc.declare_dram_parameter("idxA", [128, LA // 16], i16, isOutput=False)
    pa_d = nc.declare_dram_parameter("PA", [128, LA], P_DT, isOutput=False)
    idxB_d = nc.declare_dram_parameter("idxB", [128, LB // 16], i16, isOutput=False)
    pb_d = nc.declare_dram_parameter("PB", [128, LB], P_DT, isOutput=False)
    xout = nc.declare_dram_parameter("xout", [128, SCB * C], f32, isOutput=True)

    # ---- internal DRAM ----
    agx = [nc.dram_tensor(f"agx{l}", [128, SCB * C], SEG_DT)
           for l in range(N_LAYERS)]
    x_full = [nc.dram_tensor(f"x_full{l}", [NPAD, C], SEG_DT)
              for l in range(N_LAYERS)]
    agxe = [nc.dram_tensor(f"agxe{l}", [128, SCA * C], SEG_DT)
            for l in range(N_LAYERS)]
    xe_full = [nc.dram_tensor(f"xe_full{l}", [MPAD, C], SEG_DT)
               for l in range(N_LAYERS)]

    rg = [list(range(R))]
    qrr = [0]

    def next_q():
        q = qrr[0]
        qrr[0] = (q + 1) % 4
        return q

    RELU = mybir.ActivationFunctionType.Relu
    COPY = mybir.ActivationFunctionType.Copy

    with tile.TileContext(nc) as tc:
        with (
            tc.tile_pool(name="const", bufs=1) as cp,
            tc.tile_pool(name="work", bufs=3) as wp,
            tc.tile_pool(name="gath", bufs=2) as gp,
            tc.tile_pool(name="ptiles", bufs=2) as pp,
            tc.tile_pool(name="rows", bufs=2) as rp,
            tc.tile_pool(name="psA", bufs=2, space="PSUM") as psA,
            tc.tile_pool(name="psB", bufs=2, space="PSUM") as psB,
            tc.tile_pool(name="psC", bufs=2, space="PSUM") as psC,
            tc.tile_pool(name="psD", bufs=2, space="PSUM") as psD,
        ):
            # ---------- persistent tiles ----------
            W_IN = cp.tile([C, C], bf16)
            W2A = cp.tile([C, C], bf16)
            W2BP = cp.tile([C, C], bf16)
            W3H = cp.tile([C, C], bf16)
            BIN = cp.tile([C, 1], f32)
            B3 = cp.tile([C, 1], f32)
            DV = cp.tile([C, NO], bf16)
            XFM = cp.tile([C, NO], bf16)
            XC = cp.tile([C, NO], f32)
            IDXA = cp.tile([128, LA // 16], i16)
            IDXB = cp.tile([128, LB // 16], i16)
            IOTA = cp.tile([128, 128], f32)
            PIDX = cp.tile([128, 1], f32)
            IDENT = cp.tile([128, 128], f32)

            for t, d in [
                (W_IN, w_in), (W2A, w2a), (W2BP, w2bp), (W3H, w3h),
                (BIN, b_in), (B3, b3d), (DV, dv_d),
                (IDXA, idxA_d), (IDXB, idxB_d),
            ]:
                nc.sync.dma_start(t[:], d[:])

            nc.gpsimd.iota(IOTA[:], [[1, 128]], channel_multiplier=0,
                           allow_small_or_imprecise_dtypes=True)
            nc.gpsimd.iota(PIDX[:], [[1, 1]], channel_multiplier=1,
                           allow_small_or_imprecise_dtypes=True)
            nc.vector.tensor_scalar(IDENT[:], IOTA[:], PIDX[:], None,
                                    mybir.AluOpType.is_equal)

            # ---------- prologue: x0 = relu(x @ W_in + b_in) ----------
            for scs in _groups(SCB, GP):
                g0 = scs[0]
                gn = len(scs)
                xin = wp.tile([128, gn * C], f32, tag="xin")
                nc.sync.dma_start(xin[:], xsh[:, g0 * C: (g0 + gn) * C])
                cw = min((g0 + gn) * 128, NO) - g0 * 128
                cvt = wp.tile([C, gn * 128], f32, tag="cvt")
                nc.sync.dma_start(cvt[:, :cw],
                                  cv_d[:, g0 * 128: g0 * 128 + cw])
                rows = rp.tile([128, gn * C], SEG_DT, tag="rowsP")
                for j, s in enumerate(scs):
                    n0 = s * 128
                    ns = min(128, NO - n0)
                    ptr = psD.tile([128, 128], f32, tag="tr")
                    nc.tensor.transpose(ptr[:], xin[:, j * C: (j + 1) * C],
                                        IDENT[:])
                    xT = wp.tile([C, 128], bf16, tag="xT")
                    nc.vector.tensor_copy(xT[:], ptr[:])
                    pmm = psB.tile([C, 128], f32, tag="mm")
                    nc.tensor.matmul(pmm[:], W_IN[:], xT[:])
                    xf = wp.tile([C, 128], f32, tag="xf")
                    nc.scalar.activation(xf[:], pmm[:], RELU, bias=BIN[:, :1])
                    nc.vector.tensor_copy(XFM[:, n0: n0 + ns], xf[:, :ns])
                    nc.vector.tensor_tensor(
                        XC[:, n0: n0 + ns], xf[:, :ns],
                        cvt[:, j * 128: j * 128 + ns], mybir.AluOpType.add)
                    ptr2 = psD.tile([128, 128], f32, tag="tr")
                    nc.tensor.transpose(ptr2[:], xf[:], IDENT[:])
                    nc.scalar.activation(rows[:, j * C: (j + 1) * C], ptr2[:],
                                         COPY)
                nc.sync.dma_start(agx[0][:, g0 * C: (g0 + gn) * C], rows[:])
            nc.gpsimd.collective_compute(
                "AllGather", mybir.AluOpType.bypass, replica_groups=rg,
                ins=[agx[0][:]], outs=[x_full[0][:]],
            )

            # ---------- conv layers ----------
            for l in range(N_LAYERS):
                xf_src = x_full[l]
                # ---- phase A: XeRaw = segsum_E x[V] (row-major out) ----
                idx_off = 0
                p_off = 0
                for scs in _groups(SCA, GA):
                    g0 = scs[0]
                    gn = len(scs)
                    nAg = sum(int(nchA[s]) for s in scs)
                    nBg = sum(int(nchB[s]) for s in scs)
                    ntot = nAg + nBg
                    gt = gp.tile([128, max(ntot, 1), C], SEG_DT, tag="gt")
                    PP = pp.tile([128, max(ntot, 1) * 128], P_DT, tag="P")
                    if ntot:
                        nc.sync.dma_start(
                            PP[:, : ntot * 128],
                            pa_d[:, p_off * 128: (p_off + ntot) * 128])
                    if nAg:
                        sl = idx_off
                        nc.gpsimd.dma_gather(
                            out_ap=gt[:, 0:nAg, :], in_ap=xf_src[:],
                            idxs_ap=IDXA[:, sl // 16: (sl + nAg * 128) // 16],
                            num_idxs=nAg * 128, num_idxs_reg=nAg * 128,
                            elem_size=C, single_packet=False,
                            queue_num=next_q(),
                        )
                    if nBg:
                        sl = idx_off + nAg * 128
                        nc.gpsimd.dma_gather(
                            out_ap=gt[:, nAg:ntot, :], in_ap=xf_src[SPLIT:, :],
                            idxs_ap=IDXA[:, sl // 16: (sl + nBg * 128) // 16],
                            num_idxs=nBg * 128, num_idxs_reg=nBg * 128,
                            elem_size=C, single_packet=False,
                            queue_num=next_q(),
                        )
                    rows = rp.tile([128, gn * C], SEG_DT, tag="rowsA")
                    aoff = 0
                    boff = nAg
                    for j, s in enumerate(scs):
                        na, nb = int(nchA[s]), int(nchB[s])
                        chunks = (list(range(aoff, aoff + na))
                                  + list(range(boff, boff + nb)))
                        aoff += na
                        boff += nb
                        if not chunks:
                            nc.vector.memset(rows[:, j * C: (j + 1) * C], 0.0)
                            continue
                        ps = psA.tile([128, 128], f32, tag="seg")
                        for k, cpos in enumerate(chunks):
                            nc.tensor.matmul(
                                ps[:], PP[:, cpos * 128: (cpos + 1) * 128],
                                gt[:, cpos, :],
                                start=(k == 0), stop=(k == len(chunks) - 1),
                            )
                        nc.scalar.activation(rows[:, j * C: (j + 1) * C],
                                             ps[:], COPY)
                    nc.sync.dma_start(agxe[l][:, g0 * C: (g0 + gn) * C],
                                      rows[:])
                    idx_off += ntot * 128
                    p_off += ntot
                nc.gpsimd.collective_compute(
                    "AllGather", mybir.AluOpType.bypass, replica_groups=rg,
                    ins=[agxe[l][:]], outs=[xe_full[l][:]],
                )

                # ---- phase B ----
                last = l == N_LAYERS - 1
                row_dt = f32 if last else SEG_DT
                dst = xout if last else agx[l + 1]
                idx_off = 0
                p_off = 0
                for scs in _groups(SCB, GB):
                    g0 = scs[0]
                    gn = len(scs)
                    ntot = sum(int(nch2[s]) for s in scs)
                    gt = gp.tile([128, max(ntot, 1), C], SEG_DT, tag="gt")
                    PP = pp.tile([128, max(ntot, 1) * 128], P_DT, tag="P")
                    if ntot:
                        nc.sync.dma_start(
                            PP[:, : ntot * 128],
                            pb_d[:, p_off * 128: (p_off + ntot) * 128])
                        sl = idx_off
                        nc.gpsimd.dma_gather(
                            out_ap=gt[:, 0:ntot, :], in_ap=xe_full[l][:],
                            idxs_ap=IDXB[:, sl // 16: (sl + ntot * 128) // 16],
                            num_idxs=ntot * 128, num_idxs_reg=ntot * 128,
                            elem_size=C, single_packet=False,
                            queue_num=next_q(),
                        )
                    rows = rp.tile([128, gn * C], row_dt, tag="rowsB")
                    coff = 0
                    for j, s in enumerate(scs):
                        n0 = s * 128
                        ns = min(128, NO - n0)
                        nch = int(nch2[s])
                        ysb = wp.tile([C, 128], bf16, tag="ysb")
                        if nch:
                            ps1 = psA.tile([128, 128], f32, tag="seg")
                            for k in range(nch):
                                cpos = coff + k
                                nc.tensor.matmul(
                                    ps1[:], gt[:, cpos, :],
                                    PP[:, cpos * 128: (cpos + 1) * 128],
                                    start=(k == 0), stop=(k == nch - 1),
                                )
                            nc.scalar.activation(ysb[:], ps1[:], COPY)
                            coff += nch
                        else:
                            nc.vector.memset(ysb[:], 0.0)
                        xdeg = wp.tile([C, 128], bf16, tag="xdeg")
                        nc.vector.tensor_tensor(
                            xdeg[:, :ns], XFM[:, n0: n0 + ns],
                            DV[:, n0: n0 + ns], mybir.AluOpType.mult)
                        ps2 = psB.tile([C, 128], f32, tag="mm")
                        nc.tensor.matmul(ps2[:, :ns], W2A[:], xdeg[:, :ns],
                                         start=True, stop=False)
                        nc.tensor.matmul(ps2[:, :ns], W2BP[:], ysb[:, :ns],
                                         start=False, stop=True)
                        xmid = wp.tile([C, 128], bf16, tag="xmid")
                        nc.vector.tensor_tensor(
                            xmid[:, :ns], ps2[:, :ns], XC[:, n0: n0 + ns],
                            mybir.AluOpType.add)
                        ps3 = psC.tile([C, 128], f32, tag="out")
                        nc.tensor.matmul(ps3[:, :ns], W3H[:], xmid[:, :ns])
                        xfin = wp.tile([C, 128], f32, tag="xf")
                        nc.scalar.activation(xfin[:, :ns], ps3[:, :ns], RELU,
                                             bias=B3[:, :1])
                        if not last:
                            nc.vector.tensor_copy(XFM[:, n0: n0 + ns],
                                                  xfin[:, :ns])
                        ps4 = psD.tile([128, 128], f32, tag="tr")
                        nc.tensor.transpose(ps4[:], xfin[:], IDENT[:])
                        nc.scalar.activation(rows[:, j * C: (j + 1) * C],
                                             ps4[:], COPY)
                    nc.sync.dma_start(dst[:, g0 * C: (g0 + gn) * C], rows[:])
                    idx_off += ntot * 128
                    p_off += ntot
                if not last:
                    nc.gpsimd.collective_compute(
                        "AllGather", mybir.AluOpType.bypass, replica_groups=rg,
                        ins=[agx[l + 1][:]], outs=[x_full[l + 1][:]],
                    )
    nc.compile()
    return nc


def _get_program(V, E):
    key = (hash(V.tobytes()), hash(E.tobytes()))
    if key not in _cache:
        meta, per_core = _prepare(V, E)
        nc = _build(meta)
        _cache[key] = (nc, meta, per_core)
    return _cache[key]


def run(trace=False, trace_kwargs=None, **inputs):
    x = np.ascontiguousarray(np.asarray(inputs["x"], dtype=np.float32))
    V = np.asarray(inputs["V"]).astype(np.int64)
    E = np.asarray(inputs["E"]).astype(np.int64)
    W_in = np.asarray(inputs["W_in"], np.float32)
    b_in = np.asarray(inputs["b_in"], np.float32).reshape(C, 1)
    W1 = np.asarray(inputs["W1"], np.float64)
    b1 = np.asarray(inputs["b1"], np.float64).reshape(C)
    W2 = np.asarray(inputs["W2"], np.float64)
    b2 = np.asarray(inputs["b2"], np.float64).reshape(C)
    W3 = np.asarray(inputs["W3"], np.float32)
    b3 = np.asarray(inputs["b3"], np.float32).reshape(C, 1)
    W2a = W2[:C]
    W2b = W2[C:]
    W2bp = np.ascontiguousarray((W1 @ W2b).astype(np.float32))
    W3h = np.ascontiguousarray(0.5 * W3)
    b1w = b1 @ W2b   # [C]

    nc, meta, per_core = _get_program(V, E)

    in_maps = []
    for r in range(R):
        pc = per_core[r]
        # partition-major padded x shard
        xp = np.zeros((SCB * 128, C), np.float32)
        xp[:NO] = x[r * NO: (r + 1) * NO]
        xsh_pm = np.ascontiguousarray(
            xp.reshape(SCB, 128, C).transpose(1, 0, 2).reshape(128, SCB * C))
        cv = np.ascontiguousarray(
            (np.outer(b1w, pc["wdegV"])
             + np.outer(b2, pc["degV"])).astype(np.float32))
        dv = np.ascontiguousarray(
            np.broadcast_to(pc["degV"], (C, NO)).astype(BF_NP))
        in_maps.append({
            "xsh": xsh_pm,
            "cv": cv, "dv": dv,
            "w_in": np.ascontiguousarray(W_in.astype(BF_NP)),
            "w2a": np.ascontiguousarray(W2a.astype(BF_NP)),
            "w2bp": np.ascontiguousarray(W2bp.astype(BF_NP)),
            "w3h": np.ascontiguousarray(W3h.astype(BF_NP)),
            "b_in": b_in, "b3": b3,
            "idxA": pc["idxA"], "PA": pc["PA"],
            "idxB": pc["idxB"], "PB": pc["PB"],
        })
    res = run_bass_kernel_spmd(nc, in_maps, list(range(R)), trace=trace,
                               **(trace_kwargs or {}))
    outs = []
    for r in range(R):
        pm = res.results[r]["xout"]  # [128, SCB*C]
        rows = pm.reshape(128, SCB, C).transpose(1, 0, 2).reshape(SCB * 128, C)
        outs.append(rows[:NO])
    return np.concatenate(outs, axis=0), res


def kernel(**inputs):
    out, _ = run(**inputs)
    return out


# revision 8
# speedup vs baseline: 1.5310x; 1.5310x over previous
"""Bass/Trainium2 kernel for nn_EquivSetGNN3 (gnn_message_passing).

Math (reference): x = relu(x@W_in+b_in); x0 = x
  2 layers of: Xe = segsum_E((x@W1+b1)[V]); Xev = cat(x[V], Xe[E])@W2+b2
               Xv = segsum_V(Xev); x = relu((0.5*Xv + 0.5*x0)@W3 + b3)

Algebraic restructuring (all weight/bias work folded out of the nnz path):
  XeRaw = segsum_E x[V]                      (phase A: pure segment sum)
  SB    = segsum_V XeRaw[E]                  (phase B: pure segment sum)
  Xv    = degV (*) x @ W2a + SB @ (W1 W2b) + cvec
  cvec  = wdegV (x) (b1 W2b) + degV (x) b2   (host precomputed, [N, C])
  x'    = relu(0.5 (Xv + x0) @ W3 + b3)
where wdegV[v] = sum_{(v,e)} degE[e].

Segment sums: dma_gather of 256B f16 rows + one-hot matmuls (P in fp8) on
the TensorEngine, accumulated in PSUM per 128-segment superchunk.
Gathers are batched into few large calls (SWDGE desc-gen on the gpsimd Q7
was the baseline bottleneck at ~2.2us per call).

Sharding: nodes and edges split 8 ways (graph parallel); x and XeRaw are
AllGathered between phases; weights replicated. DRAM feature tensors use a
partition-major [128, n_sc*C] layout so all stores are single batched DMAs;
gather indices are host-remapped into that layout.
"""
import numpy as np
import ml_dtypes

import concourse.bacc as bacc
import concourse.mybir as mybir
import concourse.tile as tile
from concourse.bass_utils import run_bass_kernel_spmd

f32 = mybir.dt.float32
f16 = mybir.dt.float16
bf16 = mybir.dt.bfloat16
f8 = mybir.dt.float8e4
i16 = mybir.dt.int16

SEG_DT = f16                     # gathered-feature dtype
P_DT = f8                        # one-hot matrix dtype
P_NP = ml_dtypes.float8_e4m3
BF_NP = ml_dtypes.bfloat16

N = 50000
M = 25000
C = 128
R = 8
NO = N // R                      # 6250 nodes per core
EO = M // R                      # 3125 edges per core
SCB = (NO + 127) // 128          # 49 node superchunks per core
SCA = (EO + 127) // 128          # 25 edge superchunks per core
NPAD = R * 128 * SCB             # 50176 padded x_full rows
MPAD = R * 128 * SCA             # 25600 padded xe_full rows
SPLIT = 32768                    # int16 positive range limit for gather idxs
GA = 3                           # phase A superchunks per gather group
GB = 6                           # phase B superchunks per gather group
GP = 7                           # prologue superchunks per load group
N_LAYERS = 2

_cache = {}


def _node_gidx(v):
    """global node id -> row in partition-major padded x_full."""
    r, o = np.divmod(v, NO)
    s, p = np.divmod(o, 128)
    return r * (128 * SCB) + p * SCB + s


def _edge_gidx(e):
    """global edge id -> row in partition-major padded xe_full."""
    r, o = np.divmod(e, EO)
    s, p = np.divmod(o, 128)
    return r * (128 * SCA) + p * SCA + s


def _wrap_idx(flat):
    """[L] int -> [128, L//16] int16 (idx i at partition i%16, col i//16;
    replicated 8x across partition groups for the 8 gpsimd cores)."""
    w = flat.reshape(-1, 16).T.astype(np.int16)
    return np.ascontiguousarray(np.tile(w, (8, 1)))


def _make_P(flat):
    """[L] rel ids -> one-hot P [128, L] fp8: P[p, c*128+j] = (rel[c*128+p]==j).
    Padded entries (rel == -1) give all-zero rows."""
    rel = flat.reshape(-1, 128)          # [nch, 128] (chunk, partition)
    oh = rel[:, :, None] == np.arange(128)[None, None, :]  # [nch, p, j]
    return np.ascontiguousarray(
        oh.transpose(1, 0, 2).reshape(128, -1).astype(P_NP))


def _pad_to(arr, n, val):
    out = np.full(n, val, dtype=np.int64)
    out[: len(arr)] = arr
    return out


def _groups(n_sc, g):
    return [list(range(s, min(s + g, n_sc))) for s in range(0, n_sc, g)]


def _prepare(V, E):
    """Host-side preprocessing: sorted/sharded/padded gather index+rel arrays.

    Returns (meta, per_core). Chunk layout per gather group: all bucket-A
    chunks (superchunk-major), then all bucket-B chunks."""
    gV = _node_gidx(V)
    # ---- phase A: incidences sorted by E (edge-major) ----
    oA = np.argsort(E, kind="stable")
    Va, Ea = gV[oA], E[oA]
    e0 = np.arange(R)[:, None] * EO + np.arange(SCA)[None, :] * 128  # [R,SCA]
    e1 = np.minimum(e0 + 128, (np.arange(R)[:, None] + 1) * EO)
    lo = np.searchsorted(Ea, e0.ravel()).reshape(R, SCA)
    hi = np.searchsorted(Ea, e1.ravel()).reshape(R, SCA)

    cntA = np.zeros((R, SCA), np.int64)
    cntB = np.zeros((R, SCA), np.int64)
    for r in range(R):
        for s in range(SCA):
            seg = Va[lo[r, s]: hi[r, s]]
            nb = int((seg >= SPLIT).sum())
            cntB[r, s] = nb
            cntA[r, s] = len(seg) - nb
    nchA = (-(-cntA.max(0) // 128)).astype(np.int64)  # [SCA]
    nchB = (-(-cntB.max(0) // 128)).astype(np.int64)

    # ---- phase B: incidences sorted by V (node-major) ----
    gE = _edge_gidx(E)
    oB = np.argsort(V, kind="stable")
    Vb, Eb = V[oB], gE[oB]
    v0 = np.arange(R)[:, None] * NO + np.arange(SCB)[None, :] * 128
    v1 = np.minimum(v0 + 128, (np.arange(R)[:, None] + 1) * NO)
    lo2 = np.searchsorted(Vb, v0.ravel()).reshape(R, SCB)
    hi2 = np.searchsorted(Vb, v1.ravel()).reshape(R, SCB)
    cnt2 = hi2 - lo2
    nch2 = (-(-cnt2.max(0) // 128)).astype(np.int64)  # [SCB]

    meta = {
        "nchA": nchA.tolist(),
        "nchB": nchB.tolist(),
        "nch2": nch2.tolist(),
    }

    degE = np.bincount(E, minlength=M).astype(np.float64)
    degV = np.bincount(V, minlength=N).astype(np.float64)
    wdegV = np.zeros(N, np.float64)
    np.add.at(wdegV, V, degE[E])

    per_core = []
    for r in range(R):
        idxA_parts, relA_parts = [], []
        for scs in _groups(SCA, GA):
            aidx, arel, bidx, brel = [], [], [], []
            for s in scs:
                seg_v = Va[lo[r, s]: hi[r, s]]
                seg_e = Ea[lo[r, s]: hi[r, s]] - e0[r, s]
                mB = seg_v >= SPLIT
                la = int(nchA[s]) * 128
                aidx.append(_pad_to(seg_v[~mB], la, 0))
                arel.append(_pad_to(seg_e[~mB], la, -1))
                lb = int(nchB[s]) * 128
                bidx.append(_pad_to(seg_v[mB] - SPLIT, lb, 0))
                brel.append(_pad_to(seg_e[mB], lb, -1))
            idxA_parts += aidx + bidx
            relA_parts += arel + brel
        idxA = np.concatenate(idxA_parts)
        relA = np.concatenate(relA_parts)

        idxB_parts, relB_parts = [], []
        for scs in _groups(SCB, GB):
            for s in scs:
                seg_e = Eb[lo2[r, s]: hi2[r, s]]
                seg_v = Vb[lo2[r, s]: hi2[r, s]] - v0[r, s]
                lb = int(nch2[s]) * 128
                idxB_parts.append(_pad_to(seg_e, lb, 0))
                relB_parts.append(_pad_to(seg_v, lb, -1))
        idxB = np.concatenate(idxB_parts)
        relB = np.concatenate(relB_parts)

        per_core.append(
            {
                "idxA": _wrap_idx(idxA),
                "PA": _make_P(relA),
                "idxB": _wrap_idx(idxB),
                "PB": _make_P(relB),
                "degV": degV[r * NO: (r + 1) * NO],
                "wdegV": wdegV[r * NO: (r + 1) * NO],
            }
        )
    return meta, per_core


def _build(meta):
    nchA = meta["nchA"]
    nchB = meta["nchB"]
    nch2 = meta["nch2"]
    LA = sum(a + b for a, b in zip(nchA, nchB)) * 128
    LB = sum(nch2) * 128

    nc = bacc.Bacc("TRN2", target_bir_lowering=False, debug=False,
                   num_devices=R, num_swdge_queues=4)

    # ---- kernel I/O ----
    xsh = nc.declare_dram_parameter("xsh", [128, SCB * C], f32, isOutput=False)
    cv_d = nc.declare_dram_parameter("cv", [C, NO], f32, isOutput=False)
    dv_d = nc.declare_dram_parameter("dv", [C, NO], bf16, isOutput=False)
    w_in = nc.declare_dram_parameter("w_in", [C, C], bf16, isOutput=False)
    w2a = nc.declare_dram_parameter("w2a", [C, C], bf16, isOutput=False)
    w2bp = nc.declare_dram_parameter("w2bp", [C, C], bf16, isOutput=False)
    w3h = nc.declare_dram_parameter("w3h", [C, C], bf16, isOutput=False)
    b_in = nc.declare_dram_parameter("b_in", [C, 1], f32, isOutput=False)
    b3d = nc.declare_dram_parameter("b3", [C, 1], f32, isOutput=False)
    idxA_d = nc.declare_dram_parameter("idxA", [128, LA // 16], i16, isOutput=False)
    pa_d = nc.declare_dram_parameter("PA", [128, LA], P_DT, isOutput=False)
    idxB_d = nc.declare_dram_parameter("idxB", [128, LB // 16], i16, isOutput=False)
    pb_d = nc.declare_dram_parameter("PB", [128, LB], P_DT, isOutput=False)
    xout = nc.declare_dram_parameter("xout", [128, SCB * C], f32, isOutput=True)

    # ---- internal DRAM ----
    agx = [nc.dram_tensor(f"agx{l}", [128, SCB * C], SEG_DT)
           for l in range(N_LAYERS)]
    x_full = [nc.dram_tensor(f"x_full{l}", [NPAD, C], SEG_DT)
              for l in range(N_LAYERS)]
    agxe = [nc.dram_tensor(f"agxe{l}", [128, SCA * C], SEG_DT)
            for l in range(N_LAYERS)]
    xe_full = [nc.dram_tensor(f"xe_full{l}", [MPAD, C], SEG_DT)
               for l in range(N_LAYERS)]

    rg = [list(range(R))]
    qrr = [0]

    def next_q():
        q = qrr[0]
        qrr[0] = (q + 1) % 4
        return q

    SUBCALL = 24  # max chunks (of 128 idxs) per dma_gather sub-call

    RELU = mybir.ActivationFunctionType.Relu
    COPY = mybir.ActivationFunctionType.Copy

    with tile.TileContext(nc) as tc:
        with (
            tc.tile_pool(name="const", bufs=1) as cp,
            tc.tile_pool(name="work", bufs=3) as wp,
            tc.tile_pool(name="gath", bufs=2) as gp,
            tc.tile_pool(name="ptiles", bufs=2) as pp,
            tc.tile_pool(name="rows", bufs=2) as rp,
            tc.tile_pool(name="psA", bufs=2, space="PSUM") as psA,
            tc.tile_pool(name="psB", bufs=2, space="PSUM") as psB,
            tc.tile_pool(name="psC", bufs=2, space="PSUM") as psC,
            tc.tile_pool(name="psD", bufs=2, space="PSUM") as psD,
        ):
            # ---------- persistent tiles ----------
            W_IN = cp.tile([C, C], bf16)
            W2A = cp.tile([C, C], bf16)
            W2BP = cp.tile([C, C], bf16)
            W3H = cp.tile([C, C], bf16)
            BIN = cp.tile([C, 1], f32)
            B3 = cp.tile([C, 1], f32)
            DV = cp.tile([C, NO], bf16)
            XFM = cp.tile([C, NO], bf16)
            XC = cp.tile([C, NO], f32)
            IDXA = cp.tile([128, LA // 16], i16)
            IDXB = cp.tile([128, LB // 16], i16)
            IOTA = cp.tile([128, 128], f32)
            PIDX = cp.tile([128, 1], f32)
            IDENT = cp.tile([128, 128], f32)

            for t, d in [
                (W_IN, w_in), (W2A, w2a), (W2BP, w2bp), (W3H, w3h),
                (BIN, b_in), (B3, b3d), (DV, dv_d),
                (IDXA, idxA_d), (IDXB, idxB_d),
            ]:
                nc.sync.dma_start(t[:], d[:])

            nc.gpsimd.iota(IOTA[:], [[1, 128]], channel_multiplier=0,
                           allow_small_or_imprecise_dtypes=True)
            nc.gpsimd.iota(PIDX[:], [[1, 1]], channel_multiplier=1,
                           allow_small_or_imprecise_dtypes=True)
            nc.vector.tensor_scalar(IDENT[:], IOTA[:], PIDX[:], None,
                                    mybir.AluOpType.is_equal)

            # ---------- prologue: x0 = relu(x @ W_in + b_in) ----------
            for scs in _groups(SCB, GP):
                g0 = scs[0]
                gn = len(scs)
                xin = wp.tile([128, gn * C], f32, tag="xin")
                nc.sync.dma_start(xin[:], xsh[:, g0 * C: (g0 + gn) * C])
                cw = min((g0 + gn) * 128, NO) - g0 * 128
                cvt = wp.tile([C, gn * 128], f32, tag="cvt")
                nc.sync.dma_start(cvt[:, :cw],
                                  cv_d[:, g0 * 128: g0 * 128 + cw])
                rows = rp.tile([128, gn * C], SEG_DT, tag="rowsP")
                for j, s in enumerate(scs):
                    n0 = s * 128
                    ns = min(128, NO - n0)
                    ptr = psD.tile([128, 128], f32, tag="tr")
                    nc.tensor.transpose(ptr[:], xin[:, j * C: (j + 1) * C],
                                        IDENT[:])
                    xT = wp.tile([C, 128], bf16, tag="xT")
                    nc.vector.tensor_copy(xT[:], ptr[:])
                    pmm = psB.tile([C, 128], f32, tag="mm")
                    nc.tensor.matmul(pmm[:], W_IN[:], xT[:])
                    xf = wp.tile([C, 128], f32, tag="xf")
                    nc.scalar.activation(xf[:], pmm[:], RELU, bias=BIN[:, :1])
                    nc.vector.tensor_copy(XFM[:, n0: n0 + ns], xf[:, :ns])
                    nc.vector.tensor_tensor(
                        XC[:, n0: n0 + ns], xf[:, :ns],
                        cvt[:, j * 128: j * 128 + ns], mybir.AluOpType.add)
                    ptr2 = psD.tile([128, 128], f32, tag="tr")
                    nc.tensor.transpose(ptr2[:], xf[:], IDENT[:])
                    nc.scalar.activation(rows[:, j * C: (j + 1) * C], ptr2[:],
                                         COPY)
                nc.sync.dma_start(agx[0][:, g0 * C: (g0 + gn) * C], rows[:])
            nc.gpsimd.collective_compute(
                "AllGather", mybir.AluOpType.bypass, replica_groups=rg,
                ins=[agx[0][:]], outs=[x_full[0][:]],
            )

            # ---------- conv layers ----------
            for l in range(N_LAYERS):
                xf_src = x_full[l]
                # ---- phase A: XeRaw = segsum_E x[V] (row-major out) ----
                idx_off = 0
                p_off = 0
                for scs in _groups(SCA, GA):
                    g0 = scs[0]
                    gn = len(scs)
                    nAg = sum(int(nchA[s]) for s in scs)
                    nBg = sum(int(nchB[s]) for s in scs)
                    ntot = nAg + nBg
                    gt = gp.tile([128, max(ntot, 1), C], SEG_DT, tag="gt")
                    PP = pp.tile([128, max(ntot, 1) * 128], P_DT, tag="P")
                    if ntot:
                        nc.sync.dma_start(
                            PP[:, : ntot * 128],
                            pa_d[:, p_off * 128: (p_off + ntot) * 128])
                    def sub_gather(col0, nch_tot, src_ap, idx0):
                        done = 0
                        while done < nch_tot:
                            step = min(SUBCALL, nch_tot - done)
                            sl = idx0 + done * 128
                            nc.gpsimd.dma_gather(
                                out_ap=gt[:, col0 + done: col0 + done + step, :],
                                in_ap=src_ap,
                                idxs_ap=IDXA[:, sl // 16:
                                             (sl + step * 128) // 16],
                                num_idxs=step * 128, num_idxs_reg=step * 128,
                                elem_size=C, single_packet=False,
                                queue_num=next_q(),
                            )
                            done += step

                    if nAg:
                        sub_gather(0, nAg, xf_src[:], idx_off)
                    if nBg:
                        sub_gather(nAg, nBg, xf_src[SPLIT:, :],
                                   idx_off + nAg * 128)
                    rows = rp.tile([128, gn * C], SEG_DT, tag="rowsA")
                    aoff = 0
                    boff = nAg
                    for j, s in enumerate(scs):
                        na, nb = int(nchA[s]), int(nchB[s])
                        chunks = (list(range(aoff, aoff + na))
                                  + list(range(boff, boff + nb)))
                        aoff += na
                        boff += nb
                        if not chunks:
                            nc.vector.memset(rows[:, j * C: (j + 1) * C], 0.0)
                            continue
                        ps = psA.tile([128, 128], f32, tag="seg")
                        for k, cpos in enumerate(chunks):
                            nc.tensor.matmul(
                                ps[:], PP[:, cpos * 128: (cpos + 1) * 128],
                                gt[:, cpos, :],
                                start=(k == 0), stop=(k == len(chunks) - 1),
                            )
                        nc.scalar.activation(rows[:, j * C: (j + 1) * C],
                                             ps[:], COPY)
                    nc.sync.dma_start(agxe[l][:, g0 * C: (g0 + gn) * C],
                                      rows[:])
                    idx_off += ntot * 128
                    p_off += ntot
                nc.gpsimd.collective_compute(
                    "AllGather", mybir.AluOpType.bypass, replica_groups=rg,
                    ins=[agxe[l][:]], outs=[xe_full[l][:]],
                )

                # ---- phase B ----
                last = l == N_LAYERS - 1
                row_dt = f32 if last else SEG_DT
                dst = xout if last else agx[l + 1]
                idx_off = 0
                p_off = 0
                for scs in _groups(SCB, GB):
                    g0 = scs[0]
                    gn = len(scs)
                    ntot = sum(int(nch2[s]) for s in scs)
                    gt = gp.tile([128, max(ntot, 1), C], SEG_DT, tag="gt")
                    PP = pp.tile([128, max(ntot, 1) * 128], P_DT, tag="P")
                    if ntot:
                        nc.sync.dma_start(
                            PP[:, : ntot * 128],
                            pb_d[:, p_off * 128: (p_off + ntot) * 128])
                        done = 0
                        while done < ntot:
                            step = min(SUBCALL, ntot - done)
                            sl = idx_off + done * 128
                            nc.gpsimd.dma_gather(
                                out_ap=gt[:, done: done + step, :],
                                in_ap=xe_full[l][:],
                                idxs_ap=IDXB[:, sl // 16:
                                             (sl + step * 128) // 16],
                                num_idxs=step * 128, num_idxs_reg=step * 128,
                                elem_size=C, single_packet=False,
                                queue_num=next_q(),
                            )
                            done += step
                    rows = rp.tile([128, gn * C], row_dt, tag="rowsB")
                    coff = 0
                    for j, s in enumerate(scs):
                        n0 = s * 128
                        ns = min(128, NO - n0)
                        nch = int(nch2[s])
                        ysb = wp.tile([C, 128], bf16, tag="ysb")
                        if nch:
                            ps1 = psA.tile([128, 128], f32, tag="seg")
                            for k in range(nch):
                                cpos = coff + k
                                nc.tensor.matmul(
                                    ps1[:], gt[:, cpos, :],
                                    PP[:, cpos * 128: (cpos + 1) * 128],
                                    start=(k == 0), stop=(k == nch - 1),
                                )
                            nc.scalar.activation(ysb[:], ps1[:], COPY)
                            coff += nch
                        else:
                            nc.vector.memset(ysb[:], 0.0)
                        xdeg = wp.tile([C, 128], bf16, tag="xdeg")
                        nc.vector.tensor_tensor(
                            xdeg[:, :ns], XFM[:, n0: n0 + ns],
                            DV[:, n0: n0 + ns], mybir.AluOpType.mult)
                        ps2 = psB.tile([C, 128], f32, tag="mm")
                        nc.tensor.matmul(ps2[:, :ns], W2A[:], xdeg[:, :ns],
                                         start=True, stop=False)
                        nc.tensor.matmul(ps2[:, :ns], W2BP[:], ysb[:, :ns],
                                         start=False, stop=True)
                        xmid = wp.tile([C, 128], bf16, tag="xmid")
                        nc.vector.tensor_tensor(
                            xmid[:, :ns], ps2[:, :ns], XC[:, n0: n0 + ns],
                            mybir.AluOpType.add)
                        ps3 = psC.tile([C, 128], f32, tag="out")
                        nc.tensor.matmul(ps3[:, :ns], W3H[:], xmid[:, :ns])
                        xfin = wp.tile([C, 128], f32, tag="xf")
                        nc.scalar.activation(xfin[:, :ns], ps3[:, :ns], RELU,
                                             bias=B3[:, :1])
                        if not last:
                            nc.vector.tensor_copy(XFM[:, n0: n0 + ns],
                                                  xfin[:, :ns])
                        ps4 = psD.tile([128, 128], f32, tag="tr")
                        nc.tensor.transpose(ps4[:], xfin[:], IDENT[:])
                        nc.scalar.activation(rows[:, j * C: (j + 1) * C],
                                             ps4[:], COPY)
                    nc.sync.dma_start(dst[:, g0 * C: (g0 + gn) * C], rows[:])
                    idx_off += ntot * 128
                    p_off += ntot
                if not last:
                    nc.gpsimd.collective_compute(
                        "AllGather", mybir.AluOpType.bypass, replica_groups=rg,
                        ins=[agx[l + 1][:]], outs=[x_full[l + 1][:]],
                    )
    nc.compile()
    return nc


def _get_program(V, E):
    key = (hash(V.tobytes()), hash(E.tobytes()))
    if key not in _cache:
        meta, per_core = _prepare(V, E)
        nc = _build(meta)
        _cache[key] = (nc, meta, per_core)
    return _cache[key]


def run(trace=False, trace_kwargs=None, **inputs):
    x = np.ascontiguousarray(np.asarray(inputs["x"], dtype=np.float32))
    V = np.asarray(inputs["V"]).astype(np.int64)
    E = np.asarray(inputs["E"]).astype(np.int64)
    W_in = np.asarray(inputs["W_in"], np.float32)
    b_in = np.asarray(inputs["b_in"], np.float32).reshape(C, 1)
    W1 = np.asarray(inputs["W1"], np.float64)
    b1 = np.asarray(inputs["b1"], np.float64).reshape(C)
    W2 = np.asarray(inputs["W2"], np.float64)
    b2 = np.asarray(inputs["b2"], np.float64).reshape(C)
    W3 = np.asarray(inputs["W3"], np.float32)
    b3 = np.asarray(inputs["b3"], np.float32).reshape(C, 1)
    W2a = W2[:C]
    W2b = W2[C:]
    W2bp = np.ascontiguousarray((W1 @ W2b).astype(np.float32))
    W3h = np.ascontiguousarray(0.5 * W3)
    b1w = b1 @ W2b   # [C]

    nc, meta, per_core = _get_program(V, E)

    in_maps = []
    for r in range(R):
        pc = per_core[r]
        # partition-major padded x shard
        xp = np.zeros((SCB * 128, C), np.float32)
        xp[:NO] = x[r * NO: (r + 1) * NO]
        xsh_pm = np.ascontiguousarray(
            xp.reshape(SCB, 128, C).transpose(1, 0, 2).reshape(128, SCB * C))
        cv = np.ascontiguousarray(
            (np.outer(b1w, pc["wdegV"])
             + np.outer(b2, pc["degV"])).astype(np.float32))
        dv = np.ascontiguousarray(
            np.broadcast_to(pc["degV"], (C, NO)).astype(BF_NP))
        in_maps.append({
            "xsh": xsh_pm,
            "cv": cv, "dv": dv,
            "w_in": np.ascontiguousarray(W_in.astype(BF_NP)),
            "w2a": np.ascontiguousarray(W2a.astype(BF_NP)),
            "w2bp": np.ascontiguousarray(W2bp.astype(BF_NP)),
            "w3h": np.ascontiguousarray(W3h.astype(BF_NP)),
            "b_in": b_in, "b3": b3,
            "idxA": pc["idxA"], "PA": pc["PA"],
            "idxB": pc["idxB"], "PB": pc["PB"],
        })
    res = run_bass_kernel_spmd(nc, in_maps, list(range(R)), trace=trace,
                               **(trace_kwargs or {}))
    outs = []
    for r in range(R):
        pm = res.results[r]["xout"]  # [128, SCB*C]
        rows = pm.reshape(128, SCB, C).transpose(1, 0, 2).reshape(SCB * 128, C)
        outs.append(rows[:NO])
    return np.concatenate(outs, axis=0), res


def kernel(**inputs):
    out, _ = run(**inputs)
    return out
